# revision 11
# baseline (speedup 1.0000x reference)
"""BailingMoeV2.5 linear-attention layer on 8 Trainium2 NeuronCores.

Sharding: 2-way data parallel over batch x 4-way tensor parallel over heads
(4 heads per core). Each core computes qkv+gate projections for its heads,
qk-norm + partial RoPE, the chunked simple-GLA linear-attention scan, group
RMSNorm + sigmoid output gate, and a partial output projection; the host sums
the 4 partial outputs per batch.

Optimizations over the v1 kernel:
- Gate projection runs in fp8 (e4m3) with DoubleRow matmuls: 2 contraction
  planes per instruction halve its PE time. Scales fold into the sigmoid.
- The q-side RMSNorm scale and per-token decay exp(g(c+1)) are never applied:
  both are uniform per (token, head) so the group RMSNorm absorbs them
  exactly. A per-head rebase constant exp(g*C/2) keeps o and o^2 inside fp32
  range, and a per-(token,head) eps table restores exact eps semantics.
- The k-side 1/rms and decay scales fold into the existing att*mask STT
  (partition scalar) and the kch multiply, instead of rescaling k itself.
- RoPE is applied reading the projection PSUM directly, with the qk-norm
  elementwise weights folded into host-precomputed cos/sin tables.
- Matmuls run in bf16 with fp32 PSUM accumulation; the recurrent state is
  kept in fp32r with a bf16 shadow for the inter-chunk matmul read.
"""
import sys
sys.path.insert(0, '/opt/trn_rl_repo')
import math
import numpy as np
import ml_dtypes

import concourse.bass as bass
import concourse.bacc as bacc
import concourse.mybir as mybir
import concourse.tile as tile
from concourse.masks import make_identity
from concourse.bass_utils import run_bass_kernel_spmd

B, T, HID = 2, 4096, 2048
H, D = 16, 128
ROPE_DIM = 64
HALF = ROPE_DIM // 2      # 32
THETA = 10000.0
EPS = 1e-6
LAYER_IDX, N_LAYERS = 12, 32
C = 128                 # device chunk size
NT = T // C             # 32 token tiles per core
HL = 4                  # heads per core
NCORES = 8
KC = HID // 128         # 16 contraction chunks for qkv
KC2 = HID // 256        # 8 DoubleRow contraction pairs for the gate
SX = 16.0               # fp8 scale for hidden states
SW = 64.0               # fp8 scale for gate weights
F32, F32R, BF16 = mybir.dt.float32, mybir.dt.float32r, mybir.dt.bfloat16
FP8 = mybir.dt.float8e4
DR = mybir.MatmulPerfMode.DoubleRow
MULT, ADD = mybir.AluOpType.mult, mybir.AluOpType.add
SQUARE = mybir.ActivationFunctionType.Square
SQRT = mybir.ActivationFunctionType.Sqrt
SIGMOID = mybir.ActivationFunctionType.Sigmoid
COPY = mybir.ActivationFunctionType.Copy


def _slopes():
    start = 2.0 ** (-(2.0 ** -(math.log2(H) - 3.0)))
    s = np.array([start ** (i + 1) for i in range(H)], dtype=np.float64)
    scale = 1.0 - (LAYER_IDX - 1) / (N_LAYERS - 1) + 1e-5
    return -s * scale  # [H], negative per-step log-decay


def _bcast(handle, parts=128):
    ap = handle.ap()
    return bass.AP(tensor=ap.tensor, offset=ap.offset,
                   ap=[[0, parts]] + list(ap.ap))


def _bcast_mid(ap2d, n):
    # [P, W] -> [P, n, W] with stride-0 middle dim
    return bass.AP(tensor=ap2d.tensor, offset=ap2d.offset,
                   ap=[list(ap2d.ap[0]), [0, n], list(ap2d.ap[1])])


def build_program():
    nc = bacc.Bacc()

    hsT = nc.dram_tensor("hsT", [HID, T], BF16, kind="ExternalInput")
    x8_d = nc.dram_tensor("x8", [HID, T], FP8, kind="ExternalInput")
    w3_d = nc.dram_tensor("w3", [HID, 1536], BF16, kind="ExternalInput")
    wg8_d = nc.dram_tensor("wg8", [HID, 512], FP8, kind="ExternalInput")
    w_dT = nc.dram_tensor("w_dT", [512, 2048], BF16, kind="ExternalInput")
    rope_d = nc.dram_tensor("ropeA", [T, 8, HALF], F32, kind="ExternalInput")
    lnp_d = nc.dram_tensor("lnp", [2, ROPE_DIM], F32, kind="ExternalInput")
    ksc_d = nc.dram_tensor("ksc", [T, HL], F32, kind="ExternalInput")
    chd_d = nc.dram_tensor("chd", [HL], F32, kind="ExternalInput")
    rb_d = nc.dram_tensor("rb", [HL], F32, kind="ExternalInput")
    epsc_d = nc.dram_tensor("epsc", [C, HL], F32, kind="ExternalInput")
    gnw_d = nc.dram_tensor("gnw", [HL, D], F32, kind="ExternalInput")
    msk_d = nc.dram_tensor("msk", [C, C], F32, kind="ExternalInput")
    out_d = nc.dram_tensor("out", [T, HID], F32, kind="ExternalOutput")

    with tile.TileContext(nc) as tc:
        with tc.tile_pool(name="consts", bufs=1) as cp, \
             tc.tile_pool(name="weights", bufs=1) as wp, \
             tc.tile_pool(name="state", bufs=1) as stp, \
             tc.tile_pool(name="hin", bufs=4) as hp, \
             tc.tile_pool(name="mid", bufs=3) as mp, \
             tc.tile_pool(name="ah", bufs=3) as ap_, \
             tc.tile_pool(name="ob", bufs=3) as obp, \
             tc.tile_pool(name="ps_big", bufs=3, space="PSUM") as psb, \
             tc.tile_pool(name="ps_small", bufs=3, space="PSUM") as pss, \
             tc.tile_pool(name="ps_dense", bufs=2, space="PSUM") as psd:

            hsT_r0 = hsT.ap().rearrange("(kc kp) t -> kp kc t", kp=128)
            x8_r0 = x8_d.ap().rearrange("(kc2 two kp) t -> kp kc2 two t",
                                        kp=128, two=2)

            def load_inputs(i):
                tsl = slice(i * C, (i + 1) * C)
                ht = hp.tile([128, KC, C], BF16, tag="ht", name=f"ht{i}")
                nc.sync.dma_start(out=ht[:], in_=hsT_r0[:, :, tsl])
                x8t = hp.tile([128, KC2, 2, C], FP8, tag="x8", name=f"x8{i}")
                nc.sync.dma_start(out=x8t[:], in_=x8_r0[:, :, :, tsl])
                rc = hp.tile([C, 8, HALF], F32, tag="rc", name=f"rc{i}")
                nc.sync.dma_start(out=rc[:], in_=rope_d[tsl, :, :])
                ksc_t = hp.tile([C, HL], F32, tag="ksc", name=f"ksc{i}")
                nc.sync.dma_start(out=ksc_t[:], in_=ksc_d[tsl, :])
                return ht, x8t, rc, ksc_t

            # ---- weights / inputs, ordered so tile 0 can start ASAP ----
            w3_sb = wp.tile([128, KC, 1536], BF16)
            w3_r = w3_d.ap().rearrange("(kc kp) n -> kp kc n", kp=128)
            wg8_sb = wp.tile([128, KC2, 2, 512], FP8)
            wg8_r = wg8_d.ap().rearrange("(kc2 two kp) n -> kp kc2 two n",
                                         kp=128, two=2)
            w_dT_sb = wp.tile([128, 4, 2048], BF16)
            w_dT_r = w_dT.ap().rearrange("(kc kp) n -> kp kc n", kp=128)

            ht0 = hp.tile([128, KC, C], BF16, tag="ht", name="ht0")
            nc.sync.dma_start(out=ht0[:], in_=hsT_r0[:, :, 0:C])
            for kc in range(4):
                nc.sync.dma_start(out=w3_sb[:, kc, :], in_=w3_r[:, kc, :])

            # small constants early: attention of tile 0 needs them
            ident32 = cp.tile([128, 128], F32)
            make_identity(nc, ident32[:])
            ident_bf = cp.tile([128, 128], BF16)
            nc.vector.tensor_copy(ident_bf[:], ident32[:])
            maskT = cp.tile([C, C], F32)
            nc.sync.dma_start(out=maskT[:], in_=msk_d[:, :])
            lnp_bc = cp.tile([128, 2, ROPE_DIM], F32)
            nc.sync.dma_start(out=lnp_bc[:], in_=_bcast(lnp_d))
            gnw_bc = cp.tile([128, HL, D], F32)
            nc.sync.dma_start(out=gnw_bc[:], in_=_bcast(gnw_d))
            chd_bc = cp.tile([128, HL], F32)
            nc.sync.dma_start(out=chd_bc[:], in_=_bcast(chd_d))
            rb_bc = cp.tile([128, HL], F32)
            nc.sync.dma_start(out=rb_bc[:], in_=_bcast(rb_d))
            epsc = cp.tile([C, HL], F32)
            nc.sync.dma_start(out=epsc[:], in_=epsc_d[:, :])
            eps_t = cp.tile([128, 1], F32)
            nc.vector.memset(eps_t[:], EPS)

            x8t0 = hp.tile([128, KC2, 2, C], FP8, tag="x8", name="x80")
            nc.sync.dma_start(out=x8t0[:], in_=x8_r0[:, :, :, 0:C])
            rc0 = hp.tile([C, 8, HALF], F32, tag="rc", name="rc0")
            nc.sync.dma_start(out=rc0[:], in_=rope_d[0:C, :, :])
            ksc_t0 = hp.tile([C, HL], F32, tag="ksc", name="ksc0")
            nc.sync.dma_start(out=ksc_t0[:], in_=ksc_d[0:C, :])
            for kc in range(4, KC):
                nc.sync.dma_start(out=w3_sb[:, kc, :], in_=w3_r[:, kc, :])
            for kc2 in range(KC2):
                nc.sync.dma_start(out=wg8_sb[:, kc2, :, :],
                                  in_=wg8_r[:, kc2, :, :])
            prefetched = {0: (ht0, x8t0, rc0, ksc_t0),
                          1: load_inputs(1), 2: load_inputs(2)}
            for kc in range(4):
                nc.sync.dma_start(out=w_dT_sb[:, kc, :], in_=w_dT_r[:, kc, :])

            S_r = stp.tile([128, HL, D], F32R)
            nc.vector.memset(S_r[:].bitcast(F32), 0.0)
            S_bf = stp.tile([128, HL, D], BF16)
            nc.vector.memset(S_bf[:].bitcast(mybir.dt.uint16), 0)

            def emit_front(i):
                """Input DMA + projections + rope (+ k norm stats) for tile i."""
                tsl = slice(i * C, (i + 1) * C)
                ht, x8t, rc, ksc_t = (
                    prefetched.pop(i) if i in prefetched else load_inputs(i))

                def proj3(nb):
                    ps = psb.tile([C, 512], F32, tag="qkvg", name=f"ps{i}_{nb}")
                    for kc in range(KC):
                        nc.tensor.matmul(ps[:], ht[:, kc, :],
                                         w3_sb[:, kc, nb * 512:(nb + 1) * 512],
                                         start=(kc == 0), stop=(kc == KC - 1))
                    return ps

                def rope_finish(ps, rbase, lnrow, outb, pfx):
                    # rot pairs (r, r+32) of each head's first 64 dims; ln
                    # weights folded into the host rope tables / lnp row.
                    psh = ps[:].rearrange("c (h d) -> c h d", h=HL)
                    x0 = psh[:, :, 0:HALF]
                    x1 = psh[:, :, HALF:ROPE_DIM]
                    cq1 = _bcast_mid(rc[:, rbase + 0, :], HL)
                    sq1 = _bcast_mid(rc[:, rbase + 1, :], HL)
                    cq2 = _bcast_mid(rc[:, rbase + 2, :], HL)
                    sq2 = _bcast_mid(rc[:, rbase + 3, :], HL)
                    t0 = mp.tile([C, HL, HALF], F32, tag="t0")
                    t1 = mp.tile([C, HL, HALF], F32, tag="t1")
                    nc.vector.tensor_mul(t0[:], x0, cq1)
                    nc.vector.tensor_mul(t1[:], x1, sq1)
                    nc.vector.tensor_sub(outb[:, :, 0:HALF], t0[:], t1[:])
                    t2 = mp.tile([C, HL, HALF], F32, tag="t2")
                    t3 = mp.tile([C, HL, HALF], F32, tag="t3")
                    nc.vector.tensor_mul(t2[:], x1, cq2)
                    nc.vector.tensor_mul(t3[:], x0, sq2)
                    nc.vector.tensor_add(outb[:, :, HALF:ROPE_DIM], t2[:], t3[:])
                    nc.vector.tensor_mul(
                        outb[:, :, ROPE_DIM:D], psh[:, :, ROPE_DIM:D],
                        _bcast_mid(lnp_bc[:, lnrow, :], HL))

                qh = mp.tile([C, HL, D], BF16, tag="qh", name=f"qh{i}")
                ps_q = proj3(0)
                rope_finish(ps_q, 0, 0, qh, "q")

                kh = mp.tile([C, HL, D], BF16, tag="kh", name=f"kh{i}")
                ps_k = proj3(1)
                rope_finish(ps_k, 4, 1, kh, "k")
                # k rms stats from the raw (pre-rope) projection
                ss = mp.tile([C, HL], F32, tag="ss")
                sq = mp.tile([C, D], F32, tag="sq")
                for j in range(HL):
                    nc.scalar.activation(sq[:], ps_k[:, j * D:(j + 1) * D],
                                         SQUARE, accum_out=ss[:, j:j + 1])
                sc = mp.tile([C, HL], F32, tag="sc")
                nc.scalar.activation(sc[:], ss[:], SQRT,
                                     bias=eps_t[:], scale=1.0 / D)
                nc.vector.reciprocal(sc[:], sc[:])
                s_att = mp.tile([C, HL], F32, tag="s_att", name=f"sa{i}")
                nc.vector.tensor_mul(s_att[:], sc[:], ksc_t[:])
                s_kch = mp.tile([C, HL], F32, tag="s_kch", name=f"sk{i}")
                nc.vector.tensor_mul(s_kch[:], s_att[:], chd_bc[0:C, :])

                ps_v = proj3(2)
                v_r = mp.tile([C, HL, D], BF16, tag="v_r", name=f"v_r{i}")
                nc.scalar.copy(v_r[:], ps_v[:])

                ps_g = psb.tile([C, 512], F32, tag="qkvg", name=f"psg{i}")
                for kc2 in range(KC2):
                    nc.tensor.matmul(ps_g[:], x8t[:, kc2, :, :],
                                     wg8_sb[:, kc2, :, :],
                                     start=(kc2 == 0), stop=(kc2 == KC2 - 1),
                                     perf_mode=DR)
                g_sb = mp.tile([C, HL, D], F32, tag="g_sb", name=f"g_sb{i}")
                nc.scalar.activation(g_sb[:], ps_g[:], SIGMOID,
                                     scale=1.0 / (SX * SW))
                return dict(i=i, tsl=tsl, qh=qh, kh=kh, v_r=v_r, g_sb=g_sb,
                            s_att=s_att, s_kch=s_kch)

            def emit_back(st):
                """Attention scan + gating + dense projection for a tile."""
                i, tsl = st["i"], st["tsl"]
                qh, kh, v_r, g_sb = st["qh"], st["kh"], st["v_r"], st["g_sb"]
                s_att, s_kch = st["s_att"], st["s_kch"]

                # phase 1: feature-major q/k
                qT = [None] * HL
                kT = [None] * HL
                for j in range(HL):
                    pt_q = pss.tile([128, C], BF16, tag="sp", name=f"ptq{i}_{j}")
                    nc.tensor.transpose(pt_q[:], qh[:, j, :], ident_bf[:])
                    qT[j] = ap_.tile([128, C], BF16, tag=f"qT{j}", name=f"qT{i}_{j}")
                    nc.vector.tensor_copy(qT[j][:], pt_q[:])
                    pt_k = pss.tile([128, C], BF16, tag="sp", name=f"ptk{i}_{j}")
                    nc.tensor.transpose(pt_k[:], kh[:, j, :], ident_bf[:])
                    kT[j] = ap_.tile([128, C], BF16, tag=f"kT{j}", name=f"kT{i}_{j}")
                    nc.vector.tensor_copy(kT[j][:], pt_k[:])

                # phase 2: intra-chunk attention scores + decay-scaled k
                att = [None] * HL
                kch = [None] * HL
                for j in range(HL):
                    att_ps = pss.tile([C, C], F32, tag="sp", name=f"atp{i}_{j}")
                    nc.tensor.matmul(att_ps[:], kT[j][:], qT[j][:])
                    att[j] = ap_.tile([C, C], BF16, tag=f"att{j}", name=f"att{i}_{j}")
                    nc.vector.scalar_tensor_tensor(
                        out=att[j][:], in0=att_ps[:], scalar=s_att[:, j:j + 1],
                        in1=maskT[:], op0=MULT, op1=MULT)
                    kch[j] = ap_.tile([C, D], BF16, tag=f"kch{j}", name=f"kch{i}_{j}")
                    nc.vector.tensor_scalar_mul(kch[j][:], kh[:, j, :],
                                                s_kch[:, j:j + 1])

                # phase 3: output + state update
                o_sb = mp.tile([C, HL, D], F32, tag="o_sb", name=f"o_sb{i}")
                oss = mp.tile([C, HL], F32, tag="oss", name=f"oss{i}")
                osq = mp.tile([C, D], F32, tag="osq", name=f"osq{i}")
                for j in range(HL):
                    o_ps = pss.tile([C, D], F32, tag="sp", name=f"ops{i}_{j}")
                    nc.tensor.matmul(o_ps[:], att[j][:], v_r[:, j, :],
                                     start=True, stop=False)
                    nc.tensor.matmul(o_ps[:], qT[j][:], S_bf[:, j, :],
                                     start=False, stop=True)
                    sd_ps = pss.tile([128, D], F32, tag="sp", name=f"sdp{i}_{j}")
                    nc.tensor.matmul(sd_ps[:], kch[j][:], v_r[:, j, :])
                    nc.vector.scalar_tensor_tensor(
                        out=S_r[:, j, :], in0=S_r[:, j, :],
                        scalar=chd_bc[:, j:j + 1],
                        in1=sd_ps[:], op0=MULT, op1=ADD)
                    nc.vector.tensor_copy(S_bf[:, j, :], S_r[:, j, :])
                    nc.scalar.activation(o_sb[:, j, :], o_ps[:], COPY,
                                         scale=rb_bc[:, j:j + 1])
                    nc.scalar.activation(osq[:], o_sb[:, j, :], SQUARE,
                                         accum_out=oss[:, j:j + 1])

                # group-norm (per-token/head eps table) + gate, then transpose
                ro2 = mp.tile([C, HL], F32, tag="ro2", name=f"ro2{i}")
                nc.vector.scalar_tensor_tensor(
                    out=ro2[:], in0=oss[:], scalar=1.0 / D, in1=epsc[:],
                    op0=MULT, op1=ADD)
                ro = mp.tile([C, HL], F32, tag="ro", name=f"ro{i}")
                nc.scalar.activation(ro[:], ro2[:], SQRT)
                nc.vector.reciprocal(ro[:], ro[:])
                og_bf = mp.tile([C, HL, D], BF16, tag="og_bf", name=f"og{i}")
                ogT = mp.tile([128, HL, C], BF16, tag="ogT", name=f"ogT{i}")
                ogs = mp.tile([C, D], F32, tag="ogs", name=f"ogs{i}")
                for j in range(HL):
                    nc.vector.scalar_tensor_tensor(
                        out=ogs[:], in0=o_sb[:, j, :], scalar=ro[:, j:j + 1],
                        in1=gnw_bc[:, j, :], op0=MULT, op1=MULT)
                    nc.vector.tensor_mul(og_bf[:, j, :], ogs[:], g_sb[:, j, :])
                    pt_o = pss.tile([128, C], BF16, tag="sp", name=f"pto{i}_{j}")
                    nc.tensor.transpose(pt_o[:], og_bf[:, j, :], ident_bf[:])
                    nc.vector.tensor_copy(ogT[:, j, :], pt_o[:])

                # dense partial projection
                for nb in range(4):
                    dps = psd.tile([C, 512], F32, tag="dense", name=f"dps{i}_{nb}")
                    for kc in range(4):
                        nc.tensor.matmul(dps[:], ogT[:, kc, :],
                                         w_dT_sb[:, kc, nb * 512:(nb + 1) * 512],
                                         start=(kc == 0), stop=(kc == 3))
                    ob = obp.tile([C, 512], F32, tag="ob", name=f"ob{i}_{nb}")
                    nc.scalar.copy(ob[:], dps[:])
                    nc.sync.dma_start(out=out_d[tsl, nb * 512:(nb + 1) * 512],
                                      in_=ob[:])

            # software pipeline: keep one tile of projection work queued on
            # the PE ahead of the attention/dense stage so attention stalls
            # never starve the PE.
            pending = emit_front(0)
            for i in range(1, NT):
                nxt = emit_front(i)
                emit_back(pending)
                pending = nxt
            emit_back(pending)

    nc.finalize()
    return nc


_PROGRAM = None


def prepare_in_maps(hidden_states, w_qkv, q_ln_w, k_ln_w, g_norm_w, w_g_proj,
                    w_dense, position_ids):
    hidden_states = np.asarray(hidden_states, dtype=np.float32)
    w_qkv = np.asarray(w_qkv, dtype=np.float32)
    q_ln_w = np.asarray(q_ln_w, dtype=np.float32)
    k_ln_w = np.asarray(k_ln_w, dtype=np.float32)
    g_norm_w = np.asarray(g_norm_w, dtype=np.float32)
    w_g_proj = np.asarray(w_g_proj, dtype=np.float32)
    w_dense = np.asarray(w_dense, dtype=np.float32)
    position_ids = np.asarray(position_ids, dtype=np.int32)

    g = _slopes()  # [H] float64

    inv_freq = 1.0 / (THETA ** (np.arange(0, ROPE_DIM, 2, dtype=np.float32)
                                / ROPE_DIM))
    rope_b = []
    for b in range(B):
        freqs = position_ids[b].astype(np.float32)[:, None] * inv_freq[None, :]
        cos32 = np.cos(freqs)   # [T, 32]; emb halves share the same freqs
        sin32 = np.sin(freqs)
        rows = []
        for lnw in (q_ln_w, k_ln_w):
            rows += [cos32 * lnw[None, 0:HALF], sin32 * lnw[None, 0:HALF],
                     cos32 * lnw[None, HALF:ROPE_DIM],
                     sin32 * lnw[None, HALF:ROPE_DIM]]
        rope_b.append(np.stack(rows, axis=1).astype(np.float32))  # [T,8,32]

    lnp = np.stack([q_ln_w[ROPE_DIM:D], k_ln_w[ROPE_DIM:D]], axis=0)

    msk = np.tril(np.ones((C, C), dtype=np.float32)).T.copy()  # maskT[e,c]=c>=e
    ii = (np.arange(T) % C).astype(np.float64) + 1.0
    cc = (np.arange(C).astype(np.float64) + 1.0)

    in_maps = []
    for c in range(NCORES):
        b, hg = c // 4, c % 4
        heads = [hg * HL + j for j in range(HL)]

        hsT = np.ascontiguousarray(hidden_states[b].T)
        x8 = (hsT * SX).astype(ml_dtypes.float8_e4m3)
        hsT_bf = hsT.astype(ml_dtypes.bfloat16)

        rows = lambda w, base: np.concatenate(
            [w[base + h * D: base + (h + 1) * D] for h in heads], axis=0)
        w3 = np.concatenate([
            rows(w_qkv, 0), rows(w_qkv, H * D), rows(w_qkv, 2 * H * D)],
            axis=0)                                     # [1536, HID]
        w3_T = np.ascontiguousarray(w3.T).astype(ml_dtypes.bfloat16)
        wg = rows(w_g_proj, 0)                          # [512, HID]
        wg8_T = np.ascontiguousarray(wg.T * SW).astype(ml_dtypes.float8_e4m3)

        cols = np.concatenate([np.arange(h * D, (h + 1) * D) for h in heads])
        w_dT = np.ascontiguousarray(w_dense[:, cols].T).astype(ml_dtypes.bfloat16)

        gh = g[heads]                                    # [HL]
        ksc = np.exp(-gh[None, :] * ii[:, None])         # [T, HL]
        chd = np.exp(gh * C)
        rb = np.exp(gh * (C / 2.0))
        epsc = EPS * np.exp(2.0 * gh[None, :] * (C / 2.0 - cc[:, None]))

        in_maps.append({
            "hsT": hsT_bf,
            "x8": x8,
            "w3": w3_T,
            "wg8": wg8_T,
            "w_dT": w_dT,
            "ropeA": rope_b[b],
            "lnp": lnp,
            "ksc": ksc.astype(np.float32),
            "chd": chd.astype(np.float32),
            "rb": rb.astype(np.float32),
            "epsc": epsc.astype(np.float32),
            "gnw": np.ascontiguousarray(g_norm_w.reshape(H, D)[heads]),
            "msk": msk,
        })
    return in_maps


def kernel(hidden_states, w_qkv, q_ln_w, k_ln_w, g_norm_w, w_g_proj, w_dense,
           position_ids):
    global _PROGRAM
    in_maps = prepare_in_maps(hidden_states, w_qkv, q_ln_w, k_ln_w, g_norm_w,
                              w_g_proj, w_dense, position_ids)
    if _PROGRAM is None:
        _PROGRAM = build_program()
    res = run_bass_kernel_spmd(_PROGRAM, in_maps, list(range(NCORES)))

    out = np.zeros((B, T, HID), dtype=np.float32)
    for c in range(NCORES):
        out[c // 4] += res.results[c]["out"]
    return out


# revision 21
# speedup vs baseline: 1.0095x; 1.0095x over previous
"""BailingMoeV2.5 linear-attention layer on 8 Trainium2 NeuronCores.

Sharding: 2-way data parallel over batch x 4-way tensor parallel over heads
(4 heads per core). Each core computes qkv+gate projections for its heads,
qk-norm + partial RoPE, the chunked simple-GLA linear-attention scan, group
RMSNorm + sigmoid output gate, and a partial output projection; the host sums
the 4 partial outputs per batch.

Optimizations over the v1 kernel:
- Gate projection runs in fp8 (e4m3) with DoubleRow matmuls: 2 contraction
  planes per instruction halve its PE time. Scales fold into the sigmoid.
- The q-side RMSNorm scale and per-token decay exp(g(c+1)) are never applied:
  both are uniform per (token, head) so the group RMSNorm absorbs them
  exactly. A per-head rebase constant exp(g*C/2) keeps o and o^2 inside fp32
  range, and a per-(token,head) eps table restores exact eps semantics.
- The k-side 1/rms and decay scales fold into the existing att*mask STT
  (partition scalar) and the kch multiply, instead of rescaling k itself.
- RoPE is applied reading the projection PSUM directly, with the qk-norm
  elementwise weights folded into host-precomputed cos/sin tables.
- Matmuls run in bf16 with fp32 PSUM accumulation; the recurrent state is
  kept in fp32r with a bf16 shadow for the inter-chunk matmul read.
"""
import sys
sys.path.insert(0, '/opt/trn_rl_repo')
import math
import numpy as np
import ml_dtypes

import concourse.bass as bass
import concourse.bacc as bacc
import concourse.mybir as mybir
import concourse.tile as tile
from concourse.masks import make_identity
from concourse.bass_utils import run_bass_kernel_spmd

B, T, HID = 2, 4096, 2048
H, D = 16, 128
ROPE_DIM = 64
HALF = ROPE_DIM // 2      # 32
THETA = 10000.0
EPS = 1e-6
LAYER_IDX, N_LAYERS = 12, 32
C = 128                 # device chunk size
NT = T // C             # 32 token tiles per core
HL = 4                  # heads per core
NCORES = 8
KC = HID // 128         # 16 contraction chunks for qkv
KC2 = HID // 256        # 8 DoubleRow contraction pairs for the gate
SX = 16.0               # fp8 scale for hidden states
SW = 64.0               # fp8 scale for gate weights
F32, F32R, BF16 = mybir.dt.float32, mybir.dt.float32r, mybir.dt.bfloat16
FP8 = mybir.dt.float8e4
DR = mybir.MatmulPerfMode.DoubleRow
MULT, ADD = mybir.AluOpType.mult, mybir.AluOpType.add
SQUARE = mybir.ActivationFunctionType.Square
SQRT = mybir.ActivationFunctionType.Sqrt
SIGMOID = mybir.ActivationFunctionType.Sigmoid
COPY = mybir.ActivationFunctionType.Copy


def _slopes():
    start = 2.0 ** (-(2.0 ** -(math.log2(H) - 3.0)))
    s = np.array([start ** (i + 1) for i in range(H)], dtype=np.float64)
    scale = 1.0 - (LAYER_IDX - 1) / (N_LAYERS - 1) + 1e-5
    return -s * scale  # [H], negative per-step log-decay


def _bcast(handle, parts=128):
    ap = handle.ap()
    return bass.AP(tensor=ap.tensor, offset=ap.offset,
                   ap=[[0, parts]] + list(ap.ap))


def _bcast_mid(ap2d, n):
    # [P, W] -> [P, n, W] with stride-0 middle dim
    return bass.AP(tensor=ap2d.tensor, offset=ap2d.offset,
                   ap=[list(ap2d.ap[0]), [0, n], list(ap2d.ap[1])])


def build_program():
    nc = bacc.Bacc()

    # hsT/x8 are host-pretiled: [NT,128,KC,C] / [NT,128,KC2,2,C] flattened,
    # so each tile's DMA is 128 contiguous per-partition runs.
    hsT = nc.dram_tensor("hsT", [T, HID], BF16, kind="ExternalInput")
    x8_d = nc.dram_tensor("x8", [T, HID], FP8, kind="ExternalInput")
    w3_d = nc.dram_tensor("w3", [HID, 1536], BF16, kind="ExternalInput")
    wg8_d = nc.dram_tensor("wg8", [HID, 512], FP8, kind="ExternalInput")
    w_dT = nc.dram_tensor("w_dT", [512, 2048], BF16, kind="ExternalInput")
    rope_d = nc.dram_tensor("ropeA", [T, 8, HALF], F32, kind="ExternalInput")
    lnp_d = nc.dram_tensor("lnp", [2, ROPE_DIM], F32, kind="ExternalInput")
    ksc_d = nc.dram_tensor("ksc", [T, HL], F32, kind="ExternalInput")
    chd_d = nc.dram_tensor("chd", [HL], F32, kind="ExternalInput")
    rb_d = nc.dram_tensor("rb", [HL], F32, kind="ExternalInput")
    epsc_d = nc.dram_tensor("epsc", [C, HL], F32, kind="ExternalInput")
    gnw_d = nc.dram_tensor("gnw", [HL, D], F32, kind="ExternalInput")
    msk_d = nc.dram_tensor("msk", [C, C], F32, kind="ExternalInput")
    out_d = nc.dram_tensor("out", [T, HID], BF16, kind="ExternalOutput")

    with tile.TileContext(nc) as tc:
        with tc.tile_pool(name="consts", bufs=1) as cp, \
             tc.tile_pool(name="weights", bufs=1) as wp, \
             tc.tile_pool(name="state", bufs=1) as stp, \
             tc.tile_pool(name="hin", bufs=4) as hp, \
             tc.tile_pool(name="mid", bufs=2) as mp, \
             tc.tile_pool(name="ah", bufs=3) as ap_, \
             tc.tile_pool(name="ob", bufs=3) as obp, \
             tc.tile_pool(name="ps_big", bufs=3, space="PSUM") as psb, \
             tc.tile_pool(name="ps_small", bufs=3, space="PSUM") as pss, \
             tc.tile_pool(name="ps_dense", bufs=2, space="PSUM") as psd:

            hsT_r0 = hsT.ap().rearrange("(nt kp) (kc c) -> nt kp kc c",
                                        kp=128, c=C)
            x8_r0 = x8_d.ap().rearrange("(nt kp) (kc2 two c) -> nt kp kc2 two c",
                                        kp=128, two=2, c=C)

            def load_inputs(i):
                tsl = slice(i * C, (i + 1) * C)
                ht = hp.tile([128, KC, C], BF16, tag="ht", name=f"ht{i}")
                nc.sync.dma_start(out=ht[:], in_=hsT_r0[i])
                x8t = hp.tile([128, KC2, 2, C], FP8, tag="x8", name=f"x8{i}")
                nc.sync.dma_start(out=x8t[:], in_=x8_r0[i])
                rc = hp.tile([C, 8, HALF], F32, tag="rc", name=f"rc{i}")
                nc.sync.dma_start(out=rc[:], in_=rope_d[tsl, :, :])
                ksc_t = hp.tile([C, HL], F32, tag="ksc", name=f"ksc{i}")
                nc.sync.dma_start(out=ksc_t[:], in_=ksc_d[tsl, :])
                return ht, x8t, rc, ksc_t

            # ---- weights / inputs, ordered so tile 0 can start ASAP ----
            w3_sb = wp.tile([128, KC, 1536], BF16)
            w3_r = w3_d.ap().rearrange("(kc kp) n -> kp kc n", kp=128)
            wg8_sb = wp.tile([128, KC2, 2, 512], FP8)
            wg8_r = wg8_d.ap().rearrange("(kc2 two kp) n -> kp kc2 two n",
                                         kp=128, two=2)
            w_dT_sb = wp.tile([128, 4, 2048], BF16)
            w_dT_r = w_dT.ap().rearrange("(kc kp) n -> kp kc n", kp=128)

            ht0 = hp.tile([128, KC, C], BF16, tag="ht", name="ht0")
            nc.sync.dma_start(out=ht0[:], in_=hsT_r0[0])
            for kc in range(4):
                nc.sync.dma_start(out=w3_sb[:, kc, :], in_=w3_r[:, kc, :])

            # small constants early: attention of tile 0 needs them
            ident32 = cp.tile([128, 128], F32)
            make_identity(nc, ident32[:])
            ident_bf = cp.tile([128, 128], BF16)
            nc.vector.tensor_copy(ident_bf[:], ident32[:])
            maskT = cp.tile([C, C], F32)
            nc.sync.dma_start(out=maskT[:], in_=msk_d[:, :])
            lnp_bc = cp.tile([128, 2, ROPE_DIM], F32)
            nc.sync.dma_start(out=lnp_bc[:], in_=_bcast(lnp_d))
            gnw_bc = cp.tile([128, HL, D], F32)
            nc.sync.dma_start(out=gnw_bc[:], in_=_bcast(gnw_d))
            chd_bc = cp.tile([128, HL], F32)
            nc.sync.dma_start(out=chd_bc[:], in_=_bcast(chd_d))
            rb_bc = cp.tile([128, HL], F32)
            nc.sync.dma_start(out=rb_bc[:], in_=_bcast(rb_d))
            epsc = cp.tile([C, HL], F32)
            nc.sync.dma_start(out=epsc[:], in_=epsc_d[:, :])
            eps_t = cp.tile([128, 1], F32)
            nc.vector.memset(eps_t[:], EPS)

            x8t0 = hp.tile([128, KC2, 2, C], FP8, tag="x8", name="x80")
            nc.sync.dma_start(out=x8t0[:], in_=x8_r0[0])
            rc0 = hp.tile([C, 8, HALF], F32, tag="rc", name="rc0")
            nc.sync.dma_start(out=rc0[:], in_=rope_d[0:C, :, :])
            ksc_t0 = hp.tile([C, HL], F32, tag="ksc", name="ksc0")
            nc.sync.dma_start(out=ksc_t0[:], in_=ksc_d[0:C, :])
            for kc in range(4, KC):
                nc.sync.dma_start(out=w3_sb[:, kc, :], in_=w3_r[:, kc, :])
            for kc2 in range(KC2):
                nc.sync.dma_start(out=wg8_sb[:, kc2, :, :],
                                  in_=wg8_r[:, kc2, :, :])
            prefetched = {0: (ht0, x8t0, rc0, ksc_t0),
                          1: load_inputs(1), 2: load_inputs(2)}
            for kc in range(4):
                nc.sync.dma_start(out=w_dT_sb[:, kc, :], in_=w_dT_r[:, kc, :])

            S_r = stp.tile([128, HL, D], F32R)
            nc.vector.memset(S_r[:].bitcast(F32), 0.0)
            S_bf = stp.tile([128, HL, D], BF16)
            nc.vector.memset(S_bf[:].bitcast(mybir.dt.uint16), 0)

            def emit_front(i):
                """Input DMA + projections + rope (+ k norm stats) for tile i."""
                tsl = slice(i * C, (i + 1) * C)
                ht, x8t, rc, ksc_t = (
                    prefetched.pop(i) if i in prefetched else load_inputs(i))

                def proj3(nb):
                    ps = psb.tile([C, 512], F32, tag="qkvg", name=f"ps{i}_{nb}")
                    for kc in range(KC):
                        nc.tensor.matmul(ps[:], ht[:, kc, :],
                                         w3_sb[:, kc, nb * 512:(nb + 1) * 512],
                                         start=(kc == 0), stop=(kc == KC - 1))
                    return ps

                def rope_finish(ps, rbase, lnrow, outb, pfx):
                    # rot pairs (r, r+32) of each head's first 64 dims; ln
                    # weights folded into the host rope tables / lnp row.
                    psh = ps[:].rearrange("c (h d) -> c h d", h=HL)
                    x0 = psh[:, :, 0:HALF]
                    x1 = psh[:, :, HALF:ROPE_DIM]
                    cq1 = _bcast_mid(rc[:, rbase + 0, :], HL)
                    sq1 = _bcast_mid(rc[:, rbase + 1, :], HL)
                    cq2 = _bcast_mid(rc[:, rbase + 2, :], HL)
                    sq2 = _bcast_mid(rc[:, rbase + 3, :], HL)
                    t0 = mp.tile([C, HL, HALF], F32, tag="t0")
                    t1 = mp.tile([C, HL, HALF], F32, tag="t1")
                    nc.vector.tensor_mul(t0[:], x0, cq1)
                    nc.vector.tensor_mul(t1[:], x1, sq1)
                    nc.vector.tensor_sub(outb[:, :, 0:HALF], t0[:], t1[:])
                    t2 = mp.tile([C, HL, HALF], F32, tag="t2")
                    t3 = mp.tile([C, HL, HALF], F32, tag="t3")
                    nc.vector.tensor_mul(t2[:], x1, cq2)
                    nc.vector.tensor_mul(t3[:], x0, sq2)
                    nc.vector.tensor_add(outb[:, :, HALF:ROPE_DIM], t2[:], t3[:])
                    nc.vector.tensor_mul(
                        outb[:, :, ROPE_DIM:D], psh[:, :, ROPE_DIM:D],
                        _bcast_mid(lnp_bc[:, lnrow, :], HL))

                qh = mp.tile([C, HL, D], BF16, tag="qh", name=f"qh{i}")
                ps_q = proj3(0)
                rope_finish(ps_q, 0, 0, qh, "q")

                kh = mp.tile([C, HL, D], BF16, tag="kh", name=f"kh{i}")
                ps_k = proj3(1)
                rope_finish(ps_k, 4, 1, kh, "k")
                # k rms stats from the raw (pre-rope) projection
                ss = mp.tile([C, HL], F32, tag="ss")
                sq = mp.tile([C, D], F32, tag="sq")
                for j in range(HL):
                    nc.scalar.activation(sq[:], ps_k[:, j * D:(j + 1) * D],
                                         SQUARE, accum_out=ss[:, j:j + 1])
                sc = mp.tile([C, HL], F32, tag="sc")
                nc.scalar.activation(sc[:], ss[:], SQRT,
                                     bias=eps_t[:], scale=1.0 / D)
                nc.vector.reciprocal(sc[:], sc[:])
                s_att = mp.tile([C, HL], F32, tag="s_att", name=f"sa{i}")
                nc.vector.tensor_mul(s_att[:], sc[:], ksc_t[:])
                s_kch = mp.tile([C, HL], F32, tag="s_kch", name=f"sk{i}")
                nc.vector.tensor_mul(s_kch[:], s_att[:], chd_bc[0:C, :])

                ps_v = proj3(2)
                v_r = mp.tile([C, HL, D], BF16, tag="v_r", name=f"v_r{i}")
                nc.scalar.copy(v_r[:], ps_v[:])

                ps_g = psb.tile([C, 512], F32, tag="qkvg", name=f"psg{i}")
                for kc2 in range(KC2):
                    nc.tensor.matmul(ps_g[:], x8t[:, kc2, :, :],
                                     wg8_sb[:, kc2, :, :],
                                     start=(kc2 == 0), stop=(kc2 == KC2 - 1),
                                     perf_mode=DR)
                g_sb = mp.tile([C, HL, D], F32, tag="g_sb", name=f"g_sb{i}")
                nc.scalar.activation(g_sb[:], ps_g[:], SIGMOID,
                                     scale=1.0 / (SX * SW))
                return dict(i=i, tsl=tsl, qh=qh, kh=kh, v_r=v_r, g_sb=g_sb,
                            s_att=s_att, s_kch=s_kch)

            def emit_back(st):
                """Attention scan + gating + dense projection for a tile."""
                i, tsl = st["i"], st["tsl"]
                qh, kh, v_r, g_sb = st["qh"], st["kh"], st["v_r"], st["g_sb"]
                s_att, s_kch = st["s_att"], st["s_kch"]

                # phase 1: feature-major q/k
                qT = [None] * HL
                kT = [None] * HL
                for j in range(HL):
                    pt_q = pss.tile([128, C], BF16, tag="sp", name=f"ptq{i}_{j}")
                    nc.tensor.transpose(pt_q[:], qh[:, j, :], ident_bf[:])
                    qT[j] = ap_.tile([128, C], BF16, tag=f"qT{j}", name=f"qT{i}_{j}")
                    nc.vector.tensor_copy(qT[j][:], pt_q[:])
                    pt_k = pss.tile([128, C], BF16, tag="sp", name=f"ptk{i}_{j}")
                    nc.tensor.transpose(pt_k[:], kh[:, j, :], ident_bf[:])
                    kT[j] = ap_.tile([128, C], BF16, tag=f"kT{j}", name=f"kT{i}_{j}")
                    nc.vector.tensor_copy(kT[j][:], pt_k[:])

                # phase 2: intra-chunk attention scores + decay-scaled k
                att = [None] * HL
                kch = [None] * HL
                for j in range(HL):
                    att_ps = pss.tile([C, C], F32, tag="sp", name=f"atp{i}_{j}")
                    nc.tensor.matmul(att_ps[:], kT[j][:], qT[j][:])
                    att[j] = ap_.tile([C, C], BF16, tag=f"att{j}", name=f"att{i}_{j}")
                    nc.vector.scalar_tensor_tensor(
                        out=att[j][:], in0=att_ps[:], scalar=s_att[:, j:j + 1],
                        in1=maskT[:], op0=MULT, op1=MULT)
                    kch[j] = ap_.tile([C, D], BF16, tag=f"kch{j}", name=f"kch{i}_{j}")
                    nc.vector.tensor_scalar_mul(kch[j][:], kh[:, j, :],
                                                s_kch[:, j:j + 1])

                # phase 3: output + state update
                o_sb = mp.tile([C, HL, D], F32, tag="o_sb", name=f"o_sb{i}")
                oss = mp.tile([C, HL], F32, tag="oss", name=f"oss{i}")
                osq = mp.tile([C, D], F32, tag="osq", name=f"osq{i}")
                for j in range(HL):
                    o_ps = pss.tile([C, D], F32, tag="sp", name=f"ops{i}_{j}")
                    nc.tensor.matmul(o_ps[:], att[j][:], v_r[:, j, :],
                                     start=True, stop=False)
                    nc.tensor.matmul(o_ps[:], qT[j][:], S_bf[:, j, :],
                                     start=False, stop=True)
                    sd_ps = pss.tile([128, D], F32, tag="sp", name=f"sdp{i}_{j}")
                    nc.tensor.matmul(sd_ps[:], kch[j][:], v_r[:, j, :])
                    nc.vector.scalar_tensor_tensor(
                        out=S_r[:, j, :], in0=S_r[:, j, :],
                        scalar=chd_bc[:, j:j + 1],
                        in1=sd_ps[:], op0=MULT, op1=ADD)
                    nc.vector.tensor_copy(S_bf[:, j, :], S_r[:, j, :])
                    nc.scalar.activation(o_sb[:, j, :], o_ps[:], COPY,
                                         scale=rb_bc[:, j:j + 1])
                    nc.scalar.activation(osq[:], o_sb[:, j, :], SQUARE,
                                         accum_out=oss[:, j:j + 1])

                # group-norm (per-token/head eps table) + gate, then transpose
                ro2 = mp.tile([C, HL], F32, tag="ro2", name=f"ro2{i}")
                nc.vector.scalar_tensor_tensor(
                    out=ro2[:], in0=oss[:], scalar=1.0 / D, in1=epsc[:],
                    op0=MULT, op1=ADD)
                ro = mp.tile([C, HL], F32, tag="ro", name=f"ro{i}")
                nc.scalar.activation(ro[:], ro2[:], SQRT)
                nc.vector.reciprocal(ro[:], ro[:])
                og_bf = mp.tile([C, HL, D], BF16, tag="og_bf", name=f"og{i}")
                ogT = mp.tile([128, HL, C], BF16, tag="ogT", name=f"ogT{i}")
                ogs = mp.tile([C, D], F32, tag="ogs", name=f"ogs{i}")
                for j in range(HL):
                    nc.vector.scalar_tensor_tensor(
                        out=ogs[:], in0=o_sb[:, j, :], scalar=ro[:, j:j + 1],
                        in1=gnw_bc[:, j, :], op0=MULT, op1=MULT)
                    nc.vector.tensor_mul(og_bf[:, j, :], ogs[:], g_sb[:, j, :])
                    pt_o = pss.tile([128, C], BF16, tag="sp", name=f"pto{i}_{j}")
                    nc.tensor.transpose(pt_o[:], og_bf[:, j, :], ident_bf[:])
                    nc.vector.tensor_copy(ogT[:, j, :], pt_o[:])

                # dense partial projection
                for nb in range(4):
                    dps = psd.tile([C, 512], F32, tag="dense", name=f"dps{i}_{nb}")
                    for kc in range(4):
                        nc.tensor.matmul(dps[:], ogT[:, kc, :],
                                         w_dT_sb[:, kc, nb * 512:(nb + 1) * 512],
                                         start=(kc == 0), stop=(kc == 3))
                    ob = obp.tile([C, 512], BF16, tag="ob", name=f"ob{i}_{nb}")
                    nc.scalar.copy(ob[:], dps[:])
                    nc.sync.dma_start(out=out_d[tsl, nb * 512:(nb + 1) * 512],
                                      in_=ob[:])

            for i in range(NT):
                emit_back(emit_front(i))

    nc.finalize()
    return nc


_PROGRAM = None


def prepare_in_maps(hidden_states, w_qkv, q_ln_w, k_ln_w, g_norm_w, w_g_proj,
                    w_dense, position_ids):
    hidden_states = np.asarray(hidden_states, dtype=np.float32)
    w_qkv = np.asarray(w_qkv, dtype=np.float32)
    q_ln_w = np.asarray(q_ln_w, dtype=np.float32)
    k_ln_w = np.asarray(k_ln_w, dtype=np.float32)
    g_norm_w = np.asarray(g_norm_w, dtype=np.float32)
    w_g_proj = np.asarray(w_g_proj, dtype=np.float32)
    w_dense = np.asarray(w_dense, dtype=np.float32)
    position_ids = np.asarray(position_ids, dtype=np.int32)

    g = _slopes()  # [H] float64

    inv_freq = 1.0 / (THETA ** (np.arange(0, ROPE_DIM, 2, dtype=np.float32)
                                / ROPE_DIM))
    rope_b = []
    for b in range(B):
        freqs = position_ids[b].astype(np.float32)[:, None] * inv_freq[None, :]
        cos32 = np.cos(freqs)   # [T, 32]; emb halves share the same freqs
        sin32 = np.sin(freqs)
        rows = []
        for lnw in (q_ln_w, k_ln_w):
            rows += [cos32 * lnw[None, 0:HALF], sin32 * lnw[None, 0:HALF],
                     cos32 * lnw[None, HALF:ROPE_DIM],
                     sin32 * lnw[None, HALF:ROPE_DIM]]
        rope_b.append(np.stack(rows, axis=1).astype(np.float32))  # [T,8,32]

    lnp = np.stack([q_ln_w[ROPE_DIM:D], k_ln_w[ROPE_DIM:D]], axis=0)

    msk = np.tril(np.ones((C, C), dtype=np.float32)).T.copy()  # maskT[e,c]=c>=e
    ii = (np.arange(T) % C).astype(np.float64) + 1.0
    cc = (np.arange(C).astype(np.float64) + 1.0)

    in_maps = []
    for c in range(NCORES):
        b, hg = c // 4, c % 4
        heads = [hg * HL + j for j in range(HL)]

        hsT = np.ascontiguousarray(hidden_states[b].T)       # [HID, T]
        # pre-tiled layouts: [NT,128,KC,C] / [NT,128,KC2,2,C], flattened 2D
        hsT_bf = np.ascontiguousarray(
            hsT.reshape(KC, 128, NT, C).transpose(2, 1, 0, 3)
        ).reshape(T, HID).astype(ml_dtypes.bfloat16)
        x8 = np.ascontiguousarray(
            (hsT * SX).reshape(KC2, 2, 128, NT, C).transpose(3, 2, 0, 1, 4)
        ).reshape(T, HID).astype(ml_dtypes.float8_e4m3)

        rows = lambda w, base: np.concatenate(
            [w[base + h * D: base + (h + 1) * D] for h in heads], axis=0)
        w3 = np.concatenate([
            rows(w_qkv, 0), rows(w_qkv, H * D), rows(w_qkv, 2 * H * D)],
            axis=0)                                     # [1536, HID]
        w3_T = np.ascontiguousarray(w3.T).astype(ml_dtypes.bfloat16)
        wg = rows(w_g_proj, 0)                          # [512, HID]
        wg8_T = np.ascontiguousarray(wg.T * SW).astype(ml_dtypes.float8_e4m3)

        cols = np.concatenate([np.arange(h * D, (h + 1) * D) for h in heads])
        w_dT = np.ascontiguousarray(w_dense[:, cols].T).astype(ml_dtypes.bfloat16)

        gh = g[heads]                                    # [HL]
        ksc = np.exp(-gh[None, :] * ii[:, None])         # [T, HL]
        chd = np.exp(gh * C)
        rb = np.exp(gh * (C / 2.0))
        epsc = EPS * np.exp(2.0 * gh[None, :] * (C / 2.0 - cc[:, None]))

        in_maps.append({
            "hsT": hsT_bf,
            "x8": x8,
            "w3": w3_T,
            "wg8": wg8_T,
            "w_dT": w_dT,
            "ropeA": rope_b[b],
            "lnp": lnp,
            "ksc": ksc.astype(np.float32),
            "chd": chd.astype(np.float32),
            "rb": rb.astype(np.float32),
            "epsc": epsc.astype(np.float32),
            "gnw": np.ascontiguousarray(g_norm_w.reshape(H, D)[heads]),
            "msk": msk,
        })
    return in_maps


def kernel(hidden_states, w_qkv, q_ln_w, k_ln_w, g_norm_w, w_g_proj, w_dense,
           position_ids):
    global _PROGRAM
    in_maps = prepare_in_maps(hidden_states, w_qkv, q_ln_w, k_ln_w, g_norm_w,
                              w_g_proj, w_dense, position_ids)
    if _PROGRAM is None:
        _PROGRAM = build_program()
    res = run_bass_kernel_spmd(_PROGRAM, in_maps, list(range(NCORES)))

    out = np.zeros((B, T, HID), dtype=np.float32)
    for c in range(NCORES):
        out[c // 4] += res.results[c]["out"].astype(np.float32)
    return out
